# revision 30
# baseline (speedup 1.0000x reference)
"""DenseCL contrastive-logits kernel for 8 Trainium2 NeuronCores.

Contract: kernel(**inputs) takes the FULL unsharded inputs (named as in
setup_inputs) and returns the full [32, 65537, 50] float32 output.

Sharding:
  * The 65536-wide negative queues are split along the queue axis across
    the 8 cores (8192 columns each).
  * The match/gather stage (cosine + argmax + d_q gather) is data-parallel
    over batch: each core handles 4 of the 32 batches, then an fp16
    AllGather shares the tiny gathered d_qm tensors (401 KB total) so every
    core can compute its queue shard's logits for all 32 batches.

Precision: the match (cosine/argmax) path is kept in fp32 — the argmax
must reproduce the reference's choice exactly.  The big negative-logit
matmuls and their outputs run in fp16 (values are O(50), well inside fp16
range; ~4e-4 relative error), which makes them single-PE-pass (fp32
matmuls lower to 2 PE passes) and halves the output DMA bytes.  fp16
subnormals are flushed on the host (the PE weight path mishandles them).

Math (per batch b, t = 1/tau = 5 folded into the one-hot):
  cosT[j, i] = sum_c feat_q[b, c, j] * feat_k[b, c, i]     (PE fp32, 2 batches
                                                            packed via col tiling)
  onehotT[j, i] = t * (cosT[j, i] >= max_i cosT[j, :])      (DVE)
  onehot = onehotT^T                                        (PE transpose)
  d_qm5[d, j] = sum_i d_qT[b, i, d] * onehot[i, j]          (PE fp16)
  out_d[q, b, s] = sum_d queue_d[d, q] * d_qm5[b, d, s]     (PE fp16, q-shard)
  out_g[b, q]   = sum_d t * g_q[b, d] * queue_g[d, q]       (PE fp16, q-shard)
  pos_d[b, s]   = sum_d d_k[b, d, s] * d_qm5[b, d, s]       (fp32, local b)
  pos_g[b]      = t * sum_d g_q[b, d] * g_k[b, d]           (fp32, local b)
"""

import numpy as np

BS, DIM, S, CF, Q = 32, 128, 49, 2048, 65536
NCORES = 8
QS = Q // NCORES          # 8192 queue columns per core
BL = BS // NCORES         # 4 batches matched/gathered locally per core
BG = 4                    # batch groups in the big matmul
BPG = BS // BG            # 8 batches per group (8*49 = 392 fp32 < 1 psum bank)
CT = CF // 128            # 16 contraction chunks for the cosine
QT = QS // 128            # 64 queue tiles per core
INV_TAU = 5.0

_CACHE = {}


def _install_tile_drain_patch():
    """walrus in this container rejects instructions with >1 sync wait
    ("Too many sync wait commands" in setupSyncWait).  TileContext's
    end-of-kernel drain carries one wait per semaphore used; split them
    across a chain of single-wait drain instructions (same engine, same
    semantics)."""
    import concourse.tile as tile_mod
    import concourse.mybir as mybir
    from concourse.vector_clock import ScopedClock

    if getattr(tile_mod.TileContext, "_drain_patch_installed", False):
        return

    def _drain_and_barrier(self, tick_clock, wait_clock):
        nc = self.nc
        drain_inst = nc.sync.drain()
        wait_clock.add_sem_waits(
            drain_inst.ins, ScopedClock({None: tick_clock.global_clock})
        )
        waits = list(drain_inst.ins.sync_info.on_wait)
        if len(waits) > 1:
            drain_inst.ins.sync_info = mybir.SyncInfo(
                on_wait=waits[:1], on_update=[]
            )
            for i in range(1, len(waits)):
                extra = nc.sync.drain()
                extra.ins.sync_info = mybir.SyncInfo(
                    on_wait=waits[i : i + 1], on_update=[]
                )
        nc.all_engine_barrier()
        assert self.sems is not None
        popped = nc._tile_sem_poison_stack.pop()
        assert popped is self._sem_poison
        nc.clear_and_free_semaphores(list(self.sems.allocated().values()))
        nc.all_engine_barrier()

    tile_mod.TileContext._drain_and_barrier = _drain_and_barrier
    tile_mod.TileContext._drain_patch_installed = True


def _split_multi_waits(nc, mybir, limit=1):
    """walrus codegen here rejects instructions with more than one sync
    wait.  Hoist excess waits onto InstNoOp carriers inserted immediately
    before the offender in the same block (same engine stream => same
    semantics: all waits still execute before the instruction)."""
    n_new = 0
    for f in nc.m.functions:
        for bb in f.blocks:
            new_list = []
            changed = False
            for inst in bb.instructions:
                si = inst.sync_info
                waits = list(si.on_wait) if si is not None else []
                if len(waits) > limit:
                    for w in waits[limit:]:
                        n_new += 1
                        nop = mybir.InstNoOp(name=f"WS-{n_new}")
                        nop.engine = inst.engine
                        nop.sync_info = mybir.SyncInfo(
                            on_wait=[w], on_update=[]
                        )
                        new_list.append(nop)
                    inst.sync_info = mybir.SyncInfo(
                        on_wait=waits[:limit], on_update=list(si.on_update)
                    )
                    changed = True
                new_list.append(inst)
            if changed:
                bb.instructions = new_list


def _build():
    if "nc" in _CACHE:
        return _CACHE["nc"]

    _install_tile_drain_patch()

    import concourse.bass as bass
    import concourse.mybir as mybir
    from concourse.tile import TileContext
    from concourse.masks import make_identity

    f32 = mybir.dt.float32
    f16 = mybir.dt.float16
    X = mybir.AxisListType.X

    nc = bass.Bass()

    # ---- DRAM I/O (per-core slices prepared on the host) ----
    fqL = nc.dram_tensor("fqL", [CF, BL, S], f32, kind="ExternalInput")
    fkL = nc.dram_tensor("fkL", [CF, BL, S], f32, kind="ExternalInput")
    d_qTL = nc.dram_tensor("d_qTL", [S, BL, DIM], f16, kind="ExternalInput")
    d_kL = nc.dram_tensor("d_kL", [DIM, BL, S], f32, kind="ExternalInput")
    g_qL = nc.dram_tensor("g_qL", [BL, DIM], f32, kind="ExternalInput")
    g_kL = nc.dram_tensor("g_kL", [BL, DIM], f32, kind="ExternalInput")
    g_qT5 = nc.dram_tensor("g_qT5", [DIM, BS], f16, kind="ExternalInput")
    qg = nc.dram_tensor("qg", [DIM, QS], f16, kind="ExternalInput")
    qd = nc.dram_tensor("qd", [DIM, QS], f16, kind="ExternalInput")

    out_d = nc.dram_tensor("out_d", [QS, BS, S], f16, kind="ExternalOutput")
    out_g = nc.dram_tensor("out_g", [BS, QS], f16, kind="ExternalOutput")
    out_pos = nc.dram_tensor("out_pos", [BL, 1 + S], f32, kind="ExternalOutput")

    fqL_r = fqL.rearrange("(t p) b s -> p t b s", p=128)   # [128, CT, BL, S]
    fkL_r = fkL.rearrange("(t p) b s -> p t b s", p=128)

    with TileContext(nc) as tc:
        with (
            tc.tile_pool(name="const", bufs=1) as const_pool,
            tc.tile_pool(name="queues", bufs=1) as queue_pool,
            tc.tile_pool(name="feat", bufs=1) as feat_pool,
            tc.tile_pool(name="dqm", bufs=1) as dqm_pool,
            tc.tile_pool(name="small", bufs=3) as small_pool,
            tc.tile_pool(name="stage", bufs=6) as stage_pool,
            tc.tile_pool(name="gstage", bufs=2) as gstage_pool,
            tc.tile_pool(name="dram", bufs=1, space="DRAM") as dram_pool,
        ):
            # ---- constants / static loads ----
            ident = const_pool.tile([128, 128], f32)
            make_identity(nc, ident)
            ident16 = const_pool.tile([128, 128], f16)
            nc.vector.tensor_copy(ident16[:], ident[:])
            ones = const_pool.tile([128, 1], f32)
            nc.vector.memset(ones, 1.0)

            fq_sb = feat_pool.tile([128, CT, BL, S], f32, tag="fq")
            nc.sync.dma_start(fq_sb[:, : CT // 2], fqL_r[:, : CT // 2, :, :])
            fk_sb = feat_pool.tile([128, CT, BL, S], f32, tag="fk")
            nc.sync.dma_start(fk_sb[:, : CT // 2], fkL_r[:, : CT // 2, :, :])
            nc.sync.dma_start(fq_sb[:, CT // 2 :], fqL_r[:, CT // 2 :, :, :])
            nc.sync.dma_start(fk_sb[:, CT // 2 :], fkL_r[:, CT // 2 :, :, :])

            d_qT_sb = const_pool.tile([128, BL, DIM], f16)   # padded K
            nc.vector.memset(d_qT_sb[:], 0.0)
            nc.sync.dma_start(d_qT_sb[:S, :, :], d_qTL[:, :, :])
            d_k_sb = const_pool.tile([128, BL, S], f32)
            nc.sync.dma_start(d_k_sb[:], d_kL[:, :, :])
            g_q_sb = const_pool.tile([BL, DIM], f32)
            nc.sync.dma_start(g_q_sb[:], g_qL[:, :])
            g_k_sb = const_pool.tile([BL, DIM], f32)
            nc.sync.dma_start(g_k_sb[:], g_kL[:, :])
            qg_sb = queue_pool.tile([128, QS], f16, tag="qg")
            nc.sync.dma_start(qg_sb[:], qg[:, :])
            g_qT5_sb = const_pool.tile([128, BS], f16)
            nc.sync.dma_start(g_qT5_sb[:], g_qT5[:, :])
            qd_sb = queue_pool.tile([128, QS], f16, tag="qd")
            nc.sync.dma_start(qd_sb[:], qd[:, :])

            posd_sb = const_pool.tile([S, BL], f32)          # local pos_d [s, b]
            pos_sb = const_pool.tile([BL, 1 + S], f32)

            # ---- phase 1: match + gather for the 4 local batches ----
            dqm_loc = dqm_pool.tile([128, BL * S], f16, tag="dqml")
            p1_psum = tc.tile_pool(name="p1psum", bufs=1, space="PSUM")
            pcos_pool = p1_psum.__enter__()
            poh_pool = pdqm_pool = ppos_pool = pcos_pool
            with nc.named_scope("p1"):
                for pi in range(BL // 2):
                    bis = (2 * pi, 2 * pi + 1)
                    pcos = pcos_pool.tile(
                        [128, S], f32, tag=f"pcos{pi % 2}", name=f"pcos{pi % 2}"
                    )
                    for t in range(CT):
                        for half, bi in enumerate(bis):
                            nc.tensor.matmul(
                                pcos[64 * half : 64 * half + S, :],
                                fq_sb[:, t, bi, :],
                                fk_sb[:, t, bi, :],
                                start=(t == 0),
                                stop=(t == CT - 1),
                                tile_position=(0, 64 * half),
                                skip_group_check=True,
                            )
                    for half, bi in enumerate(bis):
                        csl = pcos[64 * half : 64 * half + S, :]
                        cmax = small_pool.tile([S, 1], f32, tag="cmax")
                        nc.vector.reduce_max(out=cmax[:], in_=csl, axis=X)
                        onehT = small_pool.tile([S, S], f16, tag="onehT")
                        nc.vector.tensor_scalar(
                            onehT[:], csl, cmax[:], INV_TAU,
                            mybir.AluOpType.is_ge, mybir.AluOpType.mult,
                        )
                        poh = poh_pool.tile([S, S], f16, tag="poh")
                        nc.tensor.transpose(poh, onehT[:], ident16[:S, :S])
                        oneh = small_pool.tile([128, S], f16, tag="oneh")
                        nc.vector.memset(oneh[:], 0.0)
                        nc.vector.tensor_copy(oneh[:S, :], poh[:])
                        pdqm = pdqm_pool.tile([128, S], f32, tag="pdqm")
                        nc.tensor.matmul(
                            pdqm, d_qT_sb[:, bi, :], oneh[:],
                            start=True, stop=True,
                        )
                        nc.vector.tensor_copy(
                            dqm_loc[:, bi * S : (bi + 1) * S], pdqm[:]
                        )

            # ---- share the 8 cores' d_qm5 (tiny, fp16); AllToAll with the
            # local block replicated 8x == AllGather, but ncfw does it as a
            # direct pairwise exchange instead of a ring ----
            with nc.named_scope("gather"):
                ag_in = dram_pool.tile(
                    [NCORES * 128, BL * S], f16, name="ag_in"
                )
                ag_out = dram_pool.tile(
                    [NCORES * 128, BL * S], f16, name="ag_out"
                )
                agi = ag_in[:].rearrange("(c p) s -> p c s", c=NCORES)
                for c in range(NCORES):
                    nc.scalar.dma_start(agi[:, c, :], dqm_loc[:])
                nc.gpsimd.collective_compute(
                    "AllToAll",
                    mybir.AluOpType.bypass,
                    replica_groups=[list(range(NCORES))],
                    ins=[ag_in[:].opt()],
                    outs=[ag_out[:].opt()],
                )

            # ---- work that hides the collective latency: pos + out_g ----
            with nc.named_scope("pos"):
                for bi in range(BL):
                    # pos_d[bi, :] = ones.T @ (d_k * d_qm5)
                    prod = small_pool.tile([128, S], f32, tag="prod")
                    nc.vector.tensor_tensor(
                        prod[:],
                        d_k_sb[:, bi, :],
                        dqm_loc[:, bi * S : (bi + 1) * S],
                        mybir.AluOpType.mult,
                    )
                    ppos = ppos_pool.tile([S, 1], f32, tag="ppos")
                    nc.tensor.matmul(
                        ppos, prod[:], ones[:], start=True, stop=True
                    )
                    nc.vector.tensor_copy(posd_sb[:, bi : bi + 1], ppos[:])
                prodg = small_pool.tile([BL, DIM], f32, tag="prodg")
                nc.vector.tensor_tensor(
                    prodg[:], g_q_sb[:], g_k_sb[:], mybir.AluOpType.mult
                )
                posg = small_pool.tile([BL, 1], f32, tag="posg")
                nc.vector.reduce_sum(out=posg[:], in_=prodg[:], axis=X)
                nc.vector.tensor_scalar_mul(pos_sb[:, 0:1], posg[:], INV_TAU)
                pposT = ppos_pool.tile([BL, S], f32, tag="pposT")
                nc.tensor.transpose(pposT, posd_sb[:], ident[:S, :S])
                nc.vector.tensor_copy(pos_sb[:, 1:], pposT[:])
                nc.sync.dma_start(out_pos[:, :], pos_sb[:])

            p1_psum.__exit__(None, None, None)
            pmm_ctx = tc.tile_pool(name="pmm", bufs=5, space="PSUM")
            pmm_pool = pmm_ctx.__enter__()
            pg_ctx = tc.tile_pool(name="pg", bufs=3, space="PSUM")
            pg_pool = pg_ctx.__enter__()

            # ---- out_g = (g_q.T * invtau).T @ queue_g shard ----
            with nc.named_scope("gphase"), tc.high_priority():
                for nt4 in range(QS // 2048):
                    gst = gstage_pool.tile([BS, 4, 512], f16, tag="gstage")
                    for k in range(4):
                        nt = nt4 * 4 + k
                        pg = pg_pool.tile([BS, 512], f32, tag="pg")
                        nc.tensor.matmul(
                            pg,
                            g_qT5_sb[:],
                            qg_sb[:, nt * 512 : (nt + 1) * 512],
                            start=True,
                            stop=True,
                        )
                        nc.vector.tensor_copy(gst[:, k], pg[:])
                    nc.sync.dma_start(
                        out_g[:, nt4 * 2048 : (nt4 + 1) * 2048],
                        gst[:].rearrange("b k n -> b (k n)"),
                    )

            # Scheduler fence: everything above (pos, out_g) must be
            # scheduled before the collective-gated loads below, so their
            # DMA-lane ticks don't entangle with the collective.
            tc.no_sync_barrier()

            # Scheduler fence: everything above (pos, out_g) must be
            # scheduled before the collective-gated loads below, so their
            # DMA-lane ticks don't entangle with the collective.
            tc.no_sync_barrier()

            # ---- unpack the gathered d_qm blocks ----
            with nc.named_scope("gather2"):
                ago = ag_out[:].rearrange("(c p) s -> p c s", c=NCORES)
                dqm_tiles = []
                for g in range(BG):
                    dt = dqm_pool.tile(
                        [128, 2, BL * S], f16, tag=f"dqm{g}", name=f"dqm{g}"
                    )
                    nc.scalar.dma_start(dt[:], ago[:, 2 * g : 2 * g + 2, :])
                    dqm_tiles.append(dt)

            # ---- phase 2: out_d over the q shard, all 32 batches per tile ----
            with nc.named_scope("p2"):
                for qt in range(QT):
                    stg = stage_pool.tile([128, BS, S], f16, tag="stage")
                    for g in range(BG):
                        pmm = pmm_pool.tile([128, BPG * S], f32, tag="pmm")
                        nc.tensor.matmul(
                            pmm,
                            qd_sb[:, qt * 128 : (qt + 1) * 128],
                            dqm_tiles[g][:].rearrange("p c s -> p (c s)"),
                            start=True,
                            stop=True,
                        )
                        src = pmm[:].rearrange("p (b s) -> p b s", b=BPG)
                        dst = stg[:, g * BPG : (g + 1) * BPG, :]
                        if g % 2 == 0:
                            nc.vector.tensor_copy(dst, src)
                        else:
                            nc.scalar.copy(dst, src)
                    nc.sync.dma_start(
                        out_d[qt * 128 : (qt + 1) * 128, :, :], stg[:]
                    )
            pg_ctx.__exit__(None, None, None)
            pmm_ctx.__exit__(None, None, None)
            pg_ctx.__exit__(None, None, None)


    _split_multi_waits(nc, mybir)

    _CACHE["nc"] = nc
    return nc


def prepare_in_maps(inputs):
    g_q = np.ascontiguousarray(inputs["g_q"], dtype=np.float32)
    g_k = np.ascontiguousarray(inputs["g_k"], dtype=np.float32)
    d_q = np.asarray(inputs["d_q"], dtype=np.float32)
    d_k = np.asarray(inputs["d_k"], dtype=np.float32)
    feat_q = np.asarray(inputs["feat_q"], dtype=np.float32)
    feat_k = np.asarray(inputs["feat_k"], dtype=np.float32)
    queue_g = np.asarray(inputs["queue_g"], dtype=np.float32)
    queue_d = np.asarray(inputs["queue_d"], dtype=np.float32)

    def to_f16(a):
        # The PE mishandles fp16 subnormals in the weight path (NaN
        # products); flush them to zero (|err| <= 6.1e-5, negligible here).
        a = a.astype(np.float16)
        a[np.abs(a) < np.float16(6.104e-5)] = np.float16(0)
        return a

    fqX = np.ascontiguousarray(feat_q.transpose(1, 0, 2))   # [CF, BS, S]
    fkX = np.ascontiguousarray(feat_k.transpose(1, 0, 2))
    d_qT = to_f16(np.ascontiguousarray(d_q.transpose(2, 0, 1)))  # [S, BS, DIM]
    d_kX = np.ascontiguousarray(d_k.transpose(1, 0, 2))     # [DIM, BS, S]
    g_qT5 = to_f16(np.ascontiguousarray(g_q.T * np.float32(INV_TAU)))
    qg16 = to_f16(queue_g)
    qd16 = to_f16(queue_d)

    in_maps = []
    for c in range(NCORES):
        sh = slice(c * QS, (c + 1) * QS)
        bl = slice(c * BL, (c + 1) * BL)
        in_maps.append(
            {
                "fqL": np.ascontiguousarray(fqX[:, bl, :]),
                "fkL": np.ascontiguousarray(fkX[:, bl, :]),
                "d_qTL": np.ascontiguousarray(d_qT[:, bl, :]),
                "d_kL": np.ascontiguousarray(d_kX[:, bl, :]),
                "g_qL": np.ascontiguousarray(g_q[bl, :]),
                "g_kL": np.ascontiguousarray(g_k[bl, :]),
                "g_qT5": g_qT5,
                "qg": np.ascontiguousarray(qg16[:, sh]),
                "qd": np.ascontiguousarray(qd16[:, sh]),
            }
        )
    return in_maps


def assemble(results) -> np.ndarray:
    out = np.empty((BS, 1 + Q, 1 + S), dtype=np.float32)
    for c in range(NCORES):
        out[c * BL : (c + 1) * BL, 0, :] = results[c]["out_pos"]
        rows = slice(1 + c * QS, 1 + (c + 1) * QS)
        out[:, rows, 0] = results[c]["out_g"].astype(np.float32)
        out[:, rows, 1:] = (
            results[c]["out_d"].transpose(1, 0, 2).astype(np.float32)
        )
    return out


def kernel(**inputs) -> np.ndarray:
    from concourse.bass_utils import run_bass_kernel_spmd

    nc = _build()
    in_maps = prepare_in_maps(inputs)
    res = run_bass_kernel_spmd(nc, in_maps, core_ids=list(range(NCORES)))
    return assemble(res.results)


# revision 31
# speedup vs baseline: 1.0303x; 1.0303x over previous
"""DenseCL contrastive-logits kernel for 8 Trainium2 NeuronCores.

Contract: kernel(**inputs) takes the FULL unsharded inputs (named as in
setup_inputs) and returns the full [32, 65537, 50] float32 output.

Sharding:
  * The 65536-wide negative queues are split along the queue axis across
    the 8 cores (8192 columns each).
  * The match/gather stage (cosine + argmax + d_q gather) is data-parallel
    over batch: each core handles 4 of the 32 batches, then an fp16
    AllGather shares the tiny gathered d_qm tensors (401 KB total) so every
    core can compute its queue shard's logits for all 32 batches.

Precision: the match (cosine/argmax) path is kept in fp32 — the argmax
must reproduce the reference's choice exactly.  The big negative-logit
matmuls and their outputs run in fp16 (values are O(50), well inside fp16
range; ~4e-4 relative error), which makes them single-PE-pass (fp32
matmuls lower to 2 PE passes) and halves the output DMA bytes.  fp16
subnormals are flushed on the host (the PE weight path mishandles them).

Math (per batch b, t = 1/tau = 5 folded into the one-hot):
  cosT[j, i] = sum_c feat_q[b, c, j] * feat_k[b, c, i]     (PE fp32, 2 batches
                                                            packed via col tiling)
  onehotT[j, i] = t * (cosT[j, i] >= max_i cosT[j, :])      (DVE)
  onehot = onehotT^T                                        (PE transpose)
  d_qm5[d, j] = sum_i d_qT[b, i, d] * onehot[i, j]          (PE fp16)
  out_d[q, b, s] = sum_d queue_d[d, q] * d_qm5[b, d, s]     (PE fp16, q-shard)
  out_g[b, q]   = sum_d t * g_q[b, d] * queue_g[d, q]       (PE fp16, q-shard)
  pos_d[b, s]   = sum_d d_k[b, d, s] * d_qm5[b, d, s]       (fp32, local b)
  pos_g[b]      = t * sum_d g_q[b, d] * g_k[b, d]           (fp32, local b)
"""

import numpy as np

BS, DIM, S, CF, Q = 32, 128, 49, 2048, 65536
NCORES = 8
QS = Q // NCORES          # 8192 queue columns per core
BL = BS // NCORES         # 4 batches matched/gathered locally per core
BG = 4                    # batch groups in the big matmul
BPG = BS // BG            # 8 batches per group (8*49 = 392 fp32 < 1 psum bank)
CT = CF // 128            # 16 contraction chunks for the cosine
QT = QS // 128            # 64 queue tiles per core
INV_TAU = 5.0

_CACHE = {}


def _install_tile_drain_patch():
    """walrus in this container rejects instructions with >1 sync wait
    ("Too many sync wait commands" in setupSyncWait).  TileContext's
    end-of-kernel drain carries one wait per semaphore used; split them
    across a chain of single-wait drain instructions (same engine, same
    semantics)."""
    import concourse.tile as tile_mod
    import concourse.mybir as mybir
    from concourse.vector_clock import ScopedClock

    if getattr(tile_mod.TileContext, "_drain_patch_installed", False):
        return

    def _drain_and_barrier(self, tick_clock, wait_clock):
        nc = self.nc
        drain_inst = nc.sync.drain()
        wait_clock.add_sem_waits(
            drain_inst.ins, ScopedClock({None: tick_clock.global_clock})
        )
        waits = list(drain_inst.ins.sync_info.on_wait)
        if len(waits) > 1:
            drain_inst.ins.sync_info = mybir.SyncInfo(
                on_wait=waits[:1], on_update=[]
            )
            for i in range(1, len(waits)):
                extra = nc.sync.drain()
                extra.ins.sync_info = mybir.SyncInfo(
                    on_wait=waits[i : i + 1], on_update=[]
                )
        nc.all_engine_barrier()
        assert self.sems is not None
        popped = nc._tile_sem_poison_stack.pop()
        assert popped is self._sem_poison
        nc.clear_and_free_semaphores(list(self.sems.allocated().values()))
        nc.all_engine_barrier()

    tile_mod.TileContext._drain_and_barrier = _drain_and_barrier
    tile_mod.TileContext._drain_patch_installed = True


def _split_multi_waits(nc, mybir, limit=1):
    """walrus codegen here rejects instructions with more than one sync
    wait.  Hoist excess waits onto InstNoOp carriers inserted immediately
    before the offender in the same block (same engine stream => same
    semantics: all waits still execute before the instruction)."""
    n_new = 0
    for f in nc.m.functions:
        for bb in f.blocks:
            new_list = []
            changed = False
            for inst in bb.instructions:
                si = inst.sync_info
                waits = list(si.on_wait) if si is not None else []
                if len(waits) > limit:
                    for w in waits[limit:]:
                        n_new += 1
                        nop = mybir.InstNoOp(name=f"WS-{n_new}")
                        nop.engine = inst.engine
                        nop.sync_info = mybir.SyncInfo(
                            on_wait=[w], on_update=[]
                        )
                        new_list.append(nop)
                    inst.sync_info = mybir.SyncInfo(
                        on_wait=waits[:limit], on_update=list(si.on_update)
                    )
                    changed = True
                new_list.append(inst)
            if changed:
                bb.instructions = new_list


def _build():
    if "nc" in _CACHE:
        return _CACHE["nc"]

    _install_tile_drain_patch()

    import concourse.bass as bass
    import concourse.mybir as mybir
    from concourse.tile import TileContext
    from concourse.masks import make_identity

    f32 = mybir.dt.float32
    f16 = mybir.dt.float16
    X = mybir.AxisListType.X

    nc = bass.Bass()

    # ---- DRAM I/O (per-core slices prepared on the host) ----
    fqL = nc.dram_tensor("fqL", [CF, BL, S], f32, kind="ExternalInput")
    fkL = nc.dram_tensor("fkL", [CF, BL, S], f32, kind="ExternalInput")
    d_qTL = nc.dram_tensor("d_qTL", [S, BL, DIM], f16, kind="ExternalInput")
    d_kL = nc.dram_tensor("d_kL", [DIM, BL, S], f32, kind="ExternalInput")
    g_qL = nc.dram_tensor("g_qL", [BL, DIM], f32, kind="ExternalInput")
    g_kL = nc.dram_tensor("g_kL", [BL, DIM], f32, kind="ExternalInput")
    g_qT5 = nc.dram_tensor("g_qT5", [DIM, BS], f16, kind="ExternalInput")
    qg = nc.dram_tensor("qg", [DIM, QS], f16, kind="ExternalInput")
    qd = nc.dram_tensor("qd", [DIM, QS], f16, kind="ExternalInput")

    out_d = nc.dram_tensor("out_d", [QS, BS, S], f16, kind="ExternalOutput")
    out_g = nc.dram_tensor("out_g", [BS, QS], f16, kind="ExternalOutput")
    out_pos = nc.dram_tensor("out_pos", [BL, 1 + S], f32, kind="ExternalOutput")

    fqL_r = fqL.rearrange("(t p) b s -> p t b s", p=128)   # [128, CT, BL, S]
    fkL_r = fkL.rearrange("(t p) b s -> p t b s", p=128)

    with TileContext(nc) as tc:
        with (
            tc.tile_pool(name="const", bufs=1) as const_pool,
            tc.tile_pool(name="queues", bufs=1) as queue_pool,
            tc.tile_pool(name="feat", bufs=1) as feat_pool,
            tc.tile_pool(name="dqm", bufs=1) as dqm_pool,
            tc.tile_pool(name="small", bufs=3) as small_pool,
            tc.tile_pool(name="stage", bufs=6) as stage_pool,
            tc.tile_pool(name="gstage", bufs=2) as gstage_pool,
            tc.tile_pool(name="dram", bufs=1, space="DRAM") as dram_pool,
        ):
            # ---- constants / static loads ----
            ident = const_pool.tile([128, 128], f32)
            make_identity(nc, ident)
            ident16 = const_pool.tile([128, 128], f16)
            nc.vector.tensor_copy(ident16[:], ident[:])
            ones = const_pool.tile([128, 1], f32)
            nc.vector.memset(ones, 1.0)

            fq_sb = feat_pool.tile([128, CT, BL, S], f32, tag="fq")
            nc.sync.dma_start(fq_sb[:, : CT // 2], fqL_r[:, : CT // 2, :, :])
            fk_sb = feat_pool.tile([128, CT, BL, S], f32, tag="fk")
            nc.sync.dma_start(fk_sb[:, : CT // 2], fkL_r[:, : CT // 2, :, :])
            nc.sync.dma_start(fq_sb[:, CT // 2 :], fqL_r[:, CT // 2 :, :, :])
            nc.sync.dma_start(fk_sb[:, CT // 2 :], fkL_r[:, CT // 2 :, :, :])

            d_qT_sb = const_pool.tile([128, BL, DIM], f16)   # padded K
            nc.vector.memset(d_qT_sb[:], 0.0)
            nc.sync.dma_start(d_qT_sb[:S, :, :], d_qTL[:, :, :])
            d_k_sb = const_pool.tile([128, BL, S], f32)
            nc.sync.dma_start(d_k_sb[:], d_kL[:, :, :])
            g_q_sb = const_pool.tile([BL, DIM], f32)
            nc.sync.dma_start(g_q_sb[:], g_qL[:, :])
            g_k_sb = const_pool.tile([BL, DIM], f32)
            nc.sync.dma_start(g_k_sb[:], g_kL[:, :])
            qg_sb = queue_pool.tile([128, QS], f16, tag="qg")
            nc.sync.dma_start(qg_sb[:], qg[:, :])
            g_qT5_sb = const_pool.tile([128, BS], f16)
            nc.sync.dma_start(g_qT5_sb[:], g_qT5[:, :])
            qd_sb = queue_pool.tile([128, QS], f16, tag="qd")
            nc.sync.dma_start(qd_sb[:], qd[:, :])

            posd_sb = const_pool.tile([S, BL], f32)          # local pos_d [s, b]
            pos_sb = const_pool.tile([BL, 1 + S], f32)

            # ---- phase 1: match + gather for the 4 local batches ----
            dqm_loc = dqm_pool.tile([128, BL * S], f16, tag="dqml")
            p1_psum = tc.tile_pool(name="p1psum", bufs=1, space="PSUM")
            pcos_pool = p1_psum.__enter__()
            poh_pool = pdqm_pool = ppos_pool = pcos_pool
            with nc.named_scope("p1"):
                # Interleave the two batch-pairs' accumulation chains
                # chunk-by-chunk: consecutive PE matmuls alternate PSUM
                # banks and column groups, hiding each matmul's drain.
                pcos_t = [
                    pcos_pool.tile([128, S], f32, tag=f"pcos{p}", name=f"pcos{p}")
                    for p in range(BL // 2)
                ]
                for t in range(CT):
                    for pi in range(BL // 2):
                        for half in range(2):
                            bi = 2 * pi + half
                            nc.tensor.matmul(
                                pcos_t[pi][64 * half : 64 * half + S, :],
                                fq_sb[:, t, bi, :],
                                fk_sb[:, t, bi, :],
                                start=(t == 0),
                                stop=(t == CT - 1),
                                tile_position=(0, 64 * half),
                                skip_group_check=True,
                            )
                for pi in range(BL // 2):
                    bis = (2 * pi, 2 * pi + 1)
                    pcos = pcos_t[pi]
                    for half, bi in enumerate(bis):
                        csl = pcos[64 * half : 64 * half + S, :]
                        cmax = small_pool.tile([S, 1], f32, tag="cmax")
                        nc.vector.reduce_max(out=cmax[:], in_=csl, axis=X)
                        onehT = small_pool.tile([S, S], f16, tag="onehT")
                        nc.vector.tensor_scalar(
                            onehT[:], csl, cmax[:], INV_TAU,
                            mybir.AluOpType.is_ge, mybir.AluOpType.mult,
                        )
                        poh = poh_pool.tile([S, S], f16, tag="poh")
                        nc.tensor.transpose(poh, onehT[:], ident16[:S, :S])
                        oneh = small_pool.tile([128, S], f16, tag="oneh")
                        nc.vector.memset(oneh[:], 0.0)
                        nc.vector.tensor_copy(oneh[:S, :], poh[:])
                        pdqm = pdqm_pool.tile([128, S], f32, tag="pdqm")
                        nc.tensor.matmul(
                            pdqm, d_qT_sb[:, bi, :], oneh[:],
                            start=True, stop=True,
                        )
                        nc.vector.tensor_copy(
                            dqm_loc[:, bi * S : (bi + 1) * S], pdqm[:]
                        )

            # ---- share the 8 cores' d_qm5 (tiny, fp16); AllToAll with the
            # local block replicated 8x == AllGather, but ncfw does it as a
            # direct pairwise exchange instead of a ring ----
            with nc.named_scope("gather"):
                ag_in = dram_pool.tile(
                    [NCORES * 128, BL * S], f16, name="ag_in"
                )
                ag_out = dram_pool.tile(
                    [NCORES * 128, BL * S], f16, name="ag_out"
                )
                agi = ag_in[:].rearrange("(c p) s -> p c s", c=NCORES)
                for c in range(NCORES):
                    nc.scalar.dma_start(agi[:, c, :], dqm_loc[:])
                nc.gpsimd.collective_compute(
                    "AllToAll",
                    mybir.AluOpType.bypass,
                    replica_groups=[list(range(NCORES))],
                    ins=[ag_in[:].opt()],
                    outs=[ag_out[:].opt()],
                )

            # ---- work that hides the collective latency: pos + out_g ----
            with nc.named_scope("pos"):
                for bi in range(BL):
                    # pos_d[bi, :] = ones.T @ (d_k * d_qm5)
                    prod = small_pool.tile([128, S], f32, tag="prod")
                    nc.vector.tensor_tensor(
                        prod[:],
                        d_k_sb[:, bi, :],
                        dqm_loc[:, bi * S : (bi + 1) * S],
                        mybir.AluOpType.mult,
                    )
                    ppos = ppos_pool.tile([S, 1], f32, tag="ppos")
                    nc.tensor.matmul(
                        ppos, prod[:], ones[:], start=True, stop=True
                    )
                    nc.vector.tensor_copy(posd_sb[:, bi : bi + 1], ppos[:])
                prodg = small_pool.tile([BL, DIM], f32, tag="prodg")
                nc.vector.tensor_tensor(
                    prodg[:], g_q_sb[:], g_k_sb[:], mybir.AluOpType.mult
                )
                posg = small_pool.tile([BL, 1], f32, tag="posg")
                nc.vector.reduce_sum(out=posg[:], in_=prodg[:], axis=X)
                nc.vector.tensor_scalar_mul(pos_sb[:, 0:1], posg[:], INV_TAU)
                pposT = ppos_pool.tile([BL, S], f32, tag="pposT")
                nc.tensor.transpose(pposT, posd_sb[:], ident[:S, :S])
                nc.vector.tensor_copy(pos_sb[:, 1:], pposT[:])
                nc.sync.dma_start(out_pos[:, :], pos_sb[:])

            p1_psum.__exit__(None, None, None)
            pmm_ctx = tc.tile_pool(name="pmm", bufs=5, space="PSUM")
            pmm_pool = pmm_ctx.__enter__()
            pg_ctx = tc.tile_pool(name="pg", bufs=3, space="PSUM")
            pg_pool = pg_ctx.__enter__()

            # ---- out_g = (g_q.T * invtau).T @ queue_g shard ----
            with nc.named_scope("gphase"), tc.high_priority():
                for nt4 in range(QS // 2048):
                    gst = gstage_pool.tile([BS, 4, 512], f16, tag="gstage")
                    for k in range(4):
                        nt = nt4 * 4 + k
                        pg = pg_pool.tile([BS, 512], f32, tag="pg")
                        nc.tensor.matmul(
                            pg,
                            g_qT5_sb[:],
                            qg_sb[:, nt * 512 : (nt + 1) * 512],
                            start=True,
                            stop=True,
                        )
                        nc.vector.tensor_copy(gst[:, k], pg[:])
                    nc.sync.dma_start(
                        out_g[:, nt4 * 2048 : (nt4 + 1) * 2048],
                        gst[:].rearrange("b k n -> b (k n)"),
                    )

            # Scheduler fence: everything above (pos, out_g) must be
            # scheduled before the collective-gated loads below, so their
            # DMA-lane ticks don't entangle with the collective.
            tc.no_sync_barrier()

            # Scheduler fence: everything above (pos, out_g) must be
            # scheduled before the collective-gated loads below, so their
            # DMA-lane ticks don't entangle with the collective.
            tc.no_sync_barrier()

            # ---- unpack the gathered d_qm blocks ----
            with nc.named_scope("gather2"):
                ago = ag_out[:].rearrange("(c p) s -> p c s", c=NCORES)
                dqm_tiles = []
                for g in range(BG):
                    dt = dqm_pool.tile(
                        [128, 2, BL * S], f16, tag=f"dqm{g}", name=f"dqm{g}"
                    )
                    nc.scalar.dma_start(dt[:], ago[:, 2 * g : 2 * g + 2, :])
                    dqm_tiles.append(dt)

            # ---- phase 2: out_d over the q shard, all 32 batches per tile ----
            with nc.named_scope("p2"):
                for qt in range(QT):
                    stg = stage_pool.tile([128, BS, S], f16, tag="stage")
                    for g in range(BG):
                        pmm = pmm_pool.tile([128, BPG * S], f32, tag="pmm")
                        nc.tensor.matmul(
                            pmm,
                            qd_sb[:, qt * 128 : (qt + 1) * 128],
                            dqm_tiles[g][:].rearrange("p c s -> p (c s)"),
                            start=True,
                            stop=True,
                        )
                        src = pmm[:].rearrange("p (b s) -> p b s", b=BPG)
                        dst = stg[:, g * BPG : (g + 1) * BPG, :]
                        if g % 2 == 0:
                            nc.vector.tensor_copy(dst, src)
                        else:
                            nc.scalar.copy(dst, src)
                    nc.sync.dma_start(
                        out_d[qt * 128 : (qt + 1) * 128, :, :], stg[:]
                    )
            pg_ctx.__exit__(None, None, None)
            pmm_ctx.__exit__(None, None, None)
            pg_ctx.__exit__(None, None, None)


    _split_multi_waits(nc, mybir)

    _CACHE["nc"] = nc
    return nc


def prepare_in_maps(inputs):
    g_q = np.ascontiguousarray(inputs["g_q"], dtype=np.float32)
    g_k = np.ascontiguousarray(inputs["g_k"], dtype=np.float32)
    d_q = np.asarray(inputs["d_q"], dtype=np.float32)
    d_k = np.asarray(inputs["d_k"], dtype=np.float32)
    feat_q = np.asarray(inputs["feat_q"], dtype=np.float32)
    feat_k = np.asarray(inputs["feat_k"], dtype=np.float32)
    queue_g = np.asarray(inputs["queue_g"], dtype=np.float32)
    queue_d = np.asarray(inputs["queue_d"], dtype=np.float32)

    def to_f16(a):
        # The PE mishandles fp16 subnormals in the weight path (NaN
        # products); flush them to zero (|err| <= 6.1e-5, negligible here).
        a = a.astype(np.float16)
        a[np.abs(a) < np.float16(6.104e-5)] = np.float16(0)
        return a

    fqX = np.ascontiguousarray(feat_q.transpose(1, 0, 2))   # [CF, BS, S]
    fkX = np.ascontiguousarray(feat_k.transpose(1, 0, 2))
    d_qT = to_f16(np.ascontiguousarray(d_q.transpose(2, 0, 1)))  # [S, BS, DIM]
    d_kX = np.ascontiguousarray(d_k.transpose(1, 0, 2))     # [DIM, BS, S]
    g_qT5 = to_f16(np.ascontiguousarray(g_q.T * np.float32(INV_TAU)))
    qg16 = to_f16(queue_g)
    qd16 = to_f16(queue_d)

    in_maps = []
    for c in range(NCORES):
        sh = slice(c * QS, (c + 1) * QS)
        bl = slice(c * BL, (c + 1) * BL)
        in_maps.append(
            {
                "fqL": np.ascontiguousarray(fqX[:, bl, :]),
                "fkL": np.ascontiguousarray(fkX[:, bl, :]),
                "d_qTL": np.ascontiguousarray(d_qT[:, bl, :]),
                "d_kL": np.ascontiguousarray(d_kX[:, bl, :]),
                "g_qL": np.ascontiguousarray(g_q[bl, :]),
                "g_kL": np.ascontiguousarray(g_k[bl, :]),
                "g_qT5": g_qT5,
                "qg": np.ascontiguousarray(qg16[:, sh]),
                "qd": np.ascontiguousarray(qd16[:, sh]),
            }
        )
    return in_maps


def assemble(results) -> np.ndarray:
    out = np.empty((BS, 1 + Q, 1 + S), dtype=np.float32)
    for c in range(NCORES):
        out[c * BL : (c + 1) * BL, 0, :] = results[c]["out_pos"]
        rows = slice(1 + c * QS, 1 + (c + 1) * QS)
        out[:, rows, 0] = results[c]["out_g"].astype(np.float32)
        out[:, rows, 1:] = (
            results[c]["out_d"].transpose(1, 0, 2).astype(np.float32)
        )
    return out


def kernel(**inputs) -> np.ndarray:
    from concourse.bass_utils import run_bass_kernel_spmd

    nc = _build()
    in_maps = prepare_in_maps(inputs)
    res = run_bass_kernel_spmd(nc, in_maps, core_ids=list(range(NCORES)))
    return assemble(res.results)
